# revision 10
# baseline (speedup 1.0000x reference)
"""Trainium2 Bass kernel for nn_CustomDeepseekDBOModel (DeepSeek-style MoE layer).

Strategy (8 NeuronCores, expert-parallel):
  * Every core receives the full token set (T=1024 is small) plus its own
    shard of the routed-expert weights (4 of 32 experts) and a TP slice of
    the shared expert (1/8 of the intermediate dim).
  * Gating (softmax + group-limited top-k) is computed on every core in
    near-fp32 precision (hi/lo bf16 split matmuls).
  * Each core gathers the tokens routed to its 4 local experts with
    `dma_gather` (no inter-core dispatch traffic at all), runs the expert
    SwiGLU MLPs in bf16, scales rows by the routing weights, and gathers
    them back per token with a second `dma_gather`.
  * Routed partial + shared-expert partial accumulate into a transposed
    [H, T] fp32 buffer; a ReduceScatter sums across cores and each core
    emits its H-chunk. The host stitches chunks and transposes.

kernel(**inputs) takes the FULL unsharded inputs and returns the full
[T, H] float32 output.
"""

from contextlib import ExitStack
from dataclasses import dataclass

import ml_dtypes
import numpy as np

import concourse.bass as bass  # noqa: F401  (kept for callers/debugging)
import concourse.mybir as mybir
import concourse.tile as tile
from concourse import bacc

F32 = mybir.dt.float32
BF16 = mybir.dt.bfloat16
I16 = mybir.dt.int16
U32 = mybir.dt.uint32
NPBF16 = ml_dtypes.bfloat16

AF = mybir.ActivationFunctionType
ALU = mybir.AluOpType
AX = mybir.AxisListType


@dataclass(frozen=True)
class Cfg:
    T: int = 1024          # tokens
    H: int = 2048          # hidden
    E: int = 32            # routed experts
    K: int = 6             # top-k
    G: int = 8             # routing groups
    TKG: int = 3           # top-k groups
    IM: int = 768          # moe intermediate
    ISH: int = 1536        # shared intermediate (n_shared * IM)
    NC: int = 8            # cores
    CAP: int = 256         # internal per-expert capacity (multiple of 128)
    SCALE: float = 16.0
    no_collective: bool = False  # replace RS with a local copy (cost model)

    @property
    def EL(self):
        return self.E // self.NC

    @property
    def TT(self):
        return self.T // 128

    @property
    def HK(self):
        return self.H // 128

    @property
    def IM2(self):
        return 2 * self.IM

    @property
    def IMK(self):
        return self.IM // 128

    @property
    def CAPC(self):
        return self.CAP // 128

    @property
    def NSLOT(self):
        return self.EL * self.CAP

    @property
    def NRANK(self):
        return self.NSLOT // 128 + 1

    @property
    def ISHL(self):
        return self.ISH // self.NC

    @property
    def HO(self):
        return self.H // self.NC


FULL = Cfg()

BIGP = 1 << 14  # mask value pushed onto invalid positions


def _chunks(n, step=128):
    out = []
    o = 0
    while o < n:
        out.append((o, min(step, n - o)))
        o += step
    return out


# ---------------------------------------------------------------------------
# device program
# ---------------------------------------------------------------------------


def build_nc(cfg: Cfg):
    c = cfg
    nc = bacc.Bacc("TRN2", target_bir_lowering=False, debug=False,
                   num_devices=c.NC)

    def inp(name, shape, dt):
        return nc.dram_tensor(name, list(shape), dt, kind="ExternalInput")

    tn = {}
    tn["xrow"] = inp("xrow", (c.T, c.H), BF16)
    tn["xhiT"] = inp("xhiT", (c.H, c.T), BF16)
    tn["xloT"] = inp("xloT", (c.H, c.T), BF16)
    tn["gwhiT"] = inp("gwhiT", (c.H, c.E), BF16)
    tn["gwloT"] = inp("gwloT", (c.H, c.E), BF16)
    tn["w13T"] = inp("w13T", (c.EL, c.H, c.IM2), BF16)
    tn["w2T"] = inp("w2T", (c.EL, c.IM, c.H), BF16)
    tn["sguT"] = inp("sguT", (c.H, 2 * c.ISHL), BF16)
    tn["sdnT"] = inp("sdnT", (c.ISHL, c.H), BF16)
    tn["c_t1"] = inp("c_t1", (128, 128), F32)
    tn["c_ones"] = inp("c_ones", (128, 128), F32)
    tn["c_ident"] = inp("c_ident", (128, 128), F32)
    tn["c_iota_tok"] = inp("c_iota_tok", (128, c.TT), F32)
    tn["c_iota_slot"] = inp("c_iota_slot", (128, c.CAP), F32)
    tn["c_iota_g"] = inp("c_iota_g", (128, c.E), F32)
    tn["c_iota_eloc"] = inp("c_iota_eloc", (128, c.EL), F32)
    tn["c_ebase"] = inp("c_ebase", (128, c.EL), F32)

    tn["out_ext"] = nc.dram_tensor("out", [c.HO, c.T], F32,
                                   kind="ExternalOutput")
    tn["rs_in"] = nc.dram_tensor("rs_in", [c.H, c.T], F32)
    tn["rs_out"] = nc.dram_tensor("rs_out", [c.HO, c.T], F32)
    tn["idxd_dram"] = nc.dram_tensor("idxd_dram", [c.NSLOT], I16)
    tn["idxc_dram"] = nc.dram_tensor("idxc_dram", [c.T * c.K], I16)

    with tile.TileContext(nc) as tc:
        _build_body(nc, tc, c, tn)
    nc.compile()
    return nc


def _build_body(nc, tc, c: Cfg, tn):
    xrow = tn["xrow"]; xhiT = tn["xhiT"]; xloT = tn["xloT"]
    gwhiT = tn["gwhiT"]; gwloT = tn["gwloT"]
    w13T = tn["w13T"]; w2T = tn["w2T"]; sguT = tn["sguT"]; sdnT = tn["sdnT"]
    rs_in = tn["rs_in"]; rs_out = tn["rs_out"]; out_ext = tn["out_ext"]
    idxd_dram = tn["idxd_dram"]; idxc_dram = tn["idxc_dram"]

    NK = c.K
    HH = c.H // 2                  # H half
    HB = HH // 128                 # h-chunks per half

    with ExitStack() as es:
        P = es.enter_context(tc.tile_pool(name="persist", bufs=1))

        def load_const(t, shape, tag):
            tl = P.tile(list(shape), F32, tag=tag)
            nc.sync.dma_start(out=tl[:], in_=t.ap())
            return tl

        t1 = load_const(tn["c_t1"], (128, 128), "t1")
        ones = load_const(tn["c_ones"], (128, 128), "ones")
        ident = load_const(tn["c_ident"], (128, 128), "ident")
        iota_tok = load_const(tn["c_iota_tok"], (128, c.TT), "iota_tok")
        iota_slot = load_const(tn["c_iota_slot"], (128, c.CAP), "iota_slot")
        iota_g = load_const(tn["c_iota_g"], (128, c.E), "iota_g")
        iota_eloc = load_const(tn["c_iota_eloc"], (128, c.EL), "iota_eloc")
        ebase = load_const(tn["c_ebase"], (128, c.EL), "ebase")

        xhiT_k = []
        for kc in range(c.HK):
            t = P.tile([128, c.T], BF16, tag=f"xhiT{kc}", name=f"xhiT{kc}")
            nc.sync.dma_start(out=t[:],
                              in_=xhiT.ap()[kc * 128:(kc + 1) * 128, :])
            xhiT_k.append(t)

        # y slot-row store: [128, NRANK, H] bf16; slot s -> (s%128, s//128)
        y_sb = P.tile([128, c.NRANK, c.H], BF16, tag="y_sb", name="y_sb")
        nc.vector.memset(y_sb[:, c.NRANK - 1, :], 0.0)

        posm_sb = P.tile([128, c.TT, c.EL], F32, tag="posm_sb", name="posm_sb")
        woh_sb = P.tile([128, c.TT, c.EL], F32, tag="woh_sb", name="woh_sb")
        wslot_sb = P.tile([128, c.EL * c.CAPC], F32, tag="wslot_sb", name="wslot_sb")
        idxd_sb = [P.tile([128, c.CAP // 16], I16, tag=f"idxd{el}", name=f"idxd{el}")
                   for el in range(c.EL)]
        idxc_sb = [P.tile([128, 128 * NK // 16], I16, tag=f"idxc{tt}", name=f"idxc{tt}")
                   for tt in range(c.TT)]

        # =================================================================
        # Phase A: gating + routing
        # =================================================================
        with tc.tile_pool(name="gate", bufs=1) as GP, \
                tc.tile_pool(name="gate2", bufs=2) as G2, \
                tc.tile_pool(name="ps_gate", bufs=2, space="PSUM") as PSG, \
                tc.tile_pool(name="ps_tp", bufs=2, space="PSUM") as PST:
            xloT_k = []
            for kc in range(c.HK):
                t = GP.tile([128, c.T], BF16, tag=f"xloT{kc}", name=f"xloT{kc}")
                nc.sync.dma_start(out=t[:],
                                  in_=xloT.ap()[kc * 128:(kc + 1) * 128, :])
                xloT_k.append(t)
            gwhi_sb = GP.tile([128, c.HK, c.E], BF16, tag="gwhi", name="gwhi")
            nc.sync.dma_start(
                out=gwhi_sb[:],
                in_=gwhiT.ap().rearrange("(k p) e -> p k e", p=128))
            gwlo_sb = GP.tile([128, c.HK, c.E], BF16, tag="gwlo", name="gwlo")
            nc.sync.dma_start(
                out=gwlo_sb[:],
                in_=gwloT.ap().rearrange("(k p) e -> p k e", p=128))

            # logits^T [E, T] in near-fp32 (hi/lo split)
            lgT = GP.tile([c.E, c.T], F32, tag="lgT", name="lgT")
            for no, nh in _chunks(c.T, 512):
                ps = PSG.tile([c.E, 512], F32, tag="lgT_ps", name="lgT_ps")
                for kc in range(c.HK):
                    pairs = [(gwhi_sb[:, kc, :], xhiT_k[kc]),
                             (gwlo_sb[:, kc, :], xhiT_k[kc]),
                             (gwhi_sb[:, kc, :], xloT_k[kc])]
                    for j, (lhsT, rhs) in enumerate(pairs):
                        nc.tensor.matmul(
                            ps[:, :nh], lhsT, rhs[:, no:no + nh],
                            start=(kc == 0 and j == 0),
                            stop=(kc == c.HK - 1 and j == 2))
                nc.scalar.copy(lgT[:, no:no + nh], ps[:, :nh])

            oh_sb = GP.tile([128, c.TT, c.EL], F32, tag="oh_sb", name="oh_sb")

            for tt in range(c.TT):
                tsl = slice(tt * 128, (tt + 1) * 128)
                lg_ps = PST.tile([128, c.E], F32, tag="lg_ps", name="lg_ps")
                nc.tensor.transpose(lg_ps[:], lgT[:, tsl],
                                    ident[:c.E, :c.E])
                lg = G2.tile([128, c.E], F32, tag="lg", name="lg")
                nc.vector.tensor_copy(lg[:], lg_ps[:])

                # softmax (fp32)
                mx = G2.tile([128, 1], F32, tag="mx", name="mx")
                nc.vector.tensor_reduce(mx[:], lg[:], AX.X, ALU.max)
                mxn = G2.tile([128, 1], F32, tag="mxn", name="mxn")
                nc.vector.tensor_scalar_mul(mxn[:], mx[:], -1.0)
                exps = G2.tile([128, c.E], F32, tag="exps", name="exps")
                sums = G2.tile([128, 1], F32, tag="sums", name="sums")
                nc.scalar.activation(exps[:], lg[:], AF.Exp, bias=mxn[:],
                                     scale=1.0, accum_out=sums[:])
                rec = G2.tile([128, 1], F32, tag="rec", name="rec")
                nc.vector.reciprocal(rec[:], sums[:])
                scores = G2.tile([128, c.E], F32, tag="scores", name="scores")
                nc.vector.tensor_scalar_mul(scores[:], exps[:], rec[:])

                # group-limited mask
                gsc = G2.tile([128, c.G], F32, tag="gsc", name="gsc")
                nc.vector.tensor_reduce(
                    gsc[:], scores[:].rearrange("p (g r) -> p g r", g=c.G),
                    AX.X, ALU.max)
                gmax = G2.tile([128, 8], F32, tag="gmax", name="gmax")
                gidx = G2.tile([128, 8], U32, tag="gidx", name="gidx")
                nc.vector.max_with_indices(gmax[:], gidx[:], gsc[:])
                gidxf = G2.tile([128, c.TKG], F32, tag="gidxf", name="gidxf")
                nc.vector.tensor_copy(gidxf[:], gidx[:, :c.TKG])
                smask = G2.tile([128, c.E], F32, tag="smask", name="smask")
                eqg = G2.tile([128, c.E], F32, tag="eqg", name="eqg")
                nc.vector.tensor_scalar(smask[:], iota_g[:], gidxf[:, 0:1],
                                        None, op0=ALU.is_equal)
                for j in range(1, c.TKG):
                    nc.vector.tensor_scalar(eqg[:], iota_g[:],
                                            gidxf[:, j:j + 1], None,
                                            op0=ALU.is_equal)
                    nc.vector.tensor_tensor(smask[:], smask[:], eqg[:],
                                            op=ALU.add)
                masked = G2.tile([128, c.E], F32, tag="masked", name="masked")
                nc.vector.tensor_tensor(masked[:], scores[:], smask[:],
                                        op=ALU.mult)

                # top-K experts (sorted top-8, take first K)
                tkv = G2.tile([128, 8], F32, tag="tkv", name="tkv")
                tki = G2.tile([128, 8], U32, tag="tki", name="tki")
                nc.vector.max_with_indices(tkv[:], tki[:], masked[:])
                tkif = G2.tile([128, NK], F32, tag="tkif", name="tkif")
                nc.vector.tensor_copy(tkif[:], tki[:, :NK])

                # local one-hot / weighted one-hot
                ohL = oh_sb[:, tt, :]
                wohL = woh_sb[:, tt, :]
                eqL = G2.tile([128, c.EL], F32, tag="eqL", name="eqL")
                weqL = G2.tile([128, c.EL], F32, tag="weqL", name="weqL")
                for k in range(NK):
                    if k == 0:
                        nc.vector.tensor_scalar(ohL, iota_eloc[:],
                                                tkif[:, 0:1], None,
                                                op0=ALU.is_equal)
                        nc.vector.tensor_scalar(wohL, ohL, tkv[:, 0:1],
                                                None, op0=ALU.mult)
                    else:
                        nc.vector.tensor_scalar(eqL[:], iota_eloc[:],
                                                tkif[:, k:k + 1], None,
                                                op0=ALU.is_equal)
                        nc.vector.tensor_tensor(ohL, ohL, eqL[:], op=ALU.add)
                        nc.vector.tensor_scalar(weqL[:], eqL[:],
                                                tkv[:, k:k + 1], None,
                                                op0=ALU.mult)
                        nc.vector.tensor_tensor(wohL, wohL, weqL[:],
                                                op=ALU.add)

                # positions: exclusive cumsum over tokens
                pos_ps = PST.tile([128, c.EL], F32, tag="pos_ps", name="pos_ps")
                nc.tensor.matmul(pos_ps[:], t1[:], ohL,
                                 start=True, stop=(tt == 0))
                for tp in range(tt):
                    nc.tensor.matmul(pos_ps[:], ones[:], oh_sb[:, tp, :],
                                     start=False, stop=(tp == tt - 1))
                pos = G2.tile([128, c.EL], F32, tag="pos", name="pos")
                nc.scalar.copy(pos[:], pos_ps[:])

                # masked positions for the slot compare
                tmp = G2.tile([128, c.EL], F32, tag="tmpA", name="tmpA")
                nc.vector.tensor_scalar(tmp[:], ohL, -float(BIGP),
                                        float(BIGP), op0=ALU.mult,
                                        op1=ALU.add)
                nc.vector.tensor_tensor(posm_sb[:, tt, :], pos[:], tmp[:],
                                        op=ALU.add)

                # combine indices
                slot = G2.tile([128, c.EL], F32, tag="slot", name="slot")
                nc.vector.tensor_tensor(slot[:], pos[:], ebase[:],
                                        op=ALU.add)
                ovf = G2.tile([128, c.EL], F32, tag="ovf", name="ovf")
                nc.vector.tensor_scalar(ovf[:], pos[:], float(c.CAP),
                                        float(BIGP), op0=ALU.is_ge,
                                        op1=ALU.mult)
                nc.vector.tensor_tensor(slot[:], slot[:], ovf[:],
                                        op=ALU.add)
                nc.vector.tensor_scalar(slot[:], slot[:], float(c.NSLOT),
                                        -float(c.NSLOT), op0=ALU.min,
                                        op1=ALU.add)
                cidx = G2.tile([128, NK], F32, tag="cidx", name="cidx")
                eqc = G2.tile([128, c.EL], F32, tag="eqc", name="eqc")
                pr = G2.tile([128, c.EL], F32, tag="pr", name="pr")
                for k in range(NK):
                    nc.vector.tensor_scalar(eqc[:], iota_eloc[:],
                                            tkif[:, k:k + 1], None,
                                            op0=ALU.is_equal)
                    nc.vector.tensor_tensor(pr[:], eqc[:], slot[:],
                                            op=ALU.mult)
                    nc.vector.tensor_reduce(cidx[:, k:k + 1], pr[:], AX.X,
                                            ALU.add)
                nc.vector.tensor_scalar_add(cidx[:], cidx[:],
                                            float(c.NSLOT))

                ct_ps = PST.tile([NK, 128], F32, tag="ct_ps", name="ct_ps")
                nc.tensor.transpose(ct_ps[:], cidx[:], ident[:])
                ct16 = G2.tile([NK, 128], I16, tag="ct16", name="ct16")
                nc.vector.tensor_copy(ct16[:], ct_ps[:])
                dst = idxc_dram.ap()[tt * 128 * NK:(tt + 1) * 128 * NK]
                nc.sync.dma_start(
                    out=dst.rearrange("(t k) -> k t", k=NK), in_=ct16[:])
                for g in range(8):
                    nc.sync.dma_start(
                        out=idxc_sb[tt][g * 16:(g + 1) * 16, :],
                        in_=dst.rearrange("(f b) -> b f", b=16))

        # =================================================================
        # Phase B: slot->token inversion per local expert
        # =================================================================
        with tc.tile_pool(name="inv", bufs=2) as IV, \
                tc.tile_pool(name="ps_ids", bufs=2, space="PSUM") as PSI, \
                tc.tile_pool(name="ps_w", bufs=2, space="PSUM") as PSW, \
                tc.tile_pool(name="ps_wt", bufs=2, space="PSUM") as PSWT:
            for el in range(c.EL):
                ids_ps = PSI.tile([1, c.CAP], F32, tag="ids_ps", name="ids_ps")
                w_ps = PSW.tile([1, c.CAP], F32, tag="w_ps", name="w_ps")
                for tt in range(c.TT):
                    m = IV.tile([128, c.CAP], F32, tag="mcomp", name="mcomp")
                    nc.vector.tensor_scalar(m[:], iota_slot[:],
                                            posm_sb[:, tt, el:el + 1], None,
                                            op0=ALU.is_equal)
                    nc.tensor.matmul(ids_ps[:], iota_tok[:, tt:tt + 1], m[:],
                                     start=(tt == 0), stop=(tt == c.TT - 1))
                    nc.tensor.matmul(w_ps[:], woh_sb[:, tt, el:el + 1], m[:],
                                     start=(tt == 0), stop=(tt == c.TT - 1))
                idr = IV.tile([1, c.CAP], F32, tag="idr", name="idr")
                nc.vector.tensor_scalar(idr[:], ids_ps[:], -1.0, 0.0,
                                        op0=ALU.add, op1=ALU.max)
                id16 = IV.tile([1, c.CAP], I16, tag="id16", name="id16")
                nc.vector.tensor_copy(id16[:], idr[:])
                dst = idxd_dram.ap()[el * c.CAP:(el + 1) * c.CAP]
                nc.sync.dma_start(out=dst, in_=id16[:])
                for g in range(8):
                    nc.sync.dma_start(
                        out=idxd_sb[el][g * 16:(g + 1) * 16, :],
                        in_=dst.rearrange("(f b) -> b f", b=16))

                wrow = IV.tile([1, c.CAP], F32, tag="wrow", name="wrow")
                nc.scalar.activation(wrow[:], w_ps[:], AF.Copy,
                                     scale=c.SCALE)
                for sc in range(c.CAPC):
                    wt_ps = PSWT.tile([128, 1], F32, tag="wt_ps", name="wt_ps")
                    nc.tensor.transpose(
                        wt_ps[:], wrow[:, sc * 128:(sc + 1) * 128],
                        ident[:1, :1])
                    rank = el * c.CAPC + sc
                    nc.vector.tensor_copy(wslot_sb[:, rank:rank + 1],
                                          wt_ps[:])

        # =================================================================
        # Phase C: dispatch gather + expert MLPs
        # =================================================================
        with tc.tile_pool(name="w13p", bufs=c.HK + 4) as W13, \
                tc.tile_pool(name="w2p", bufs=c.IMK + 2) as W2P, \
                tc.tile_pool(name="xgp", bufs=2) as XGP, \
                tc.tile_pool(name="actp", bufs=2) as ACTP, \
                tc.tile_pool(name="sgp", bufs=2) as SGP, \
                tc.tile_pool(name="ps_gu", bufs=3, space="PSUM") as PSGU, \
                tc.tile_pool(name="ps_y", bufs=2, space="PSUM") as PSY:
            for el in range(c.EL):
                xg = XGP.tile([128, c.HK, c.CAP], BF16, tag="xg", name="xg")
                nc.gpsimd.dma_gather(
                    out_ap=xg[:], in_ap=xrow.ap(), idxs_ap=idxd_sb[el][:],
                    num_idxs=c.CAP, num_idxs_reg=c.CAP, elem_size=c.H,
                    transpose=True)

                w13k = []
                for kc in range(c.HK):
                    t = W13.tile([128, c.IM2], BF16, tag="w13t", name="w13t")
                    nc.sync.dma_start(
                        out=t[:],
                        in_=w13T.ap()[el, kc * 128:(kc + 1) * 128, :])
                    w13k.append(t)

                actT = ACTP.tile([128, c.IMK, c.CAP], BF16, tag="actT", name="actT")
                for mg in range(c.IMK):
                    gps = PSGU.tile([128, 512], F32, tag="gu_ps", name="gu_ps")
                    ups = PSGU.tile([128, 512], F32, tag="gu_ps", name="gu_ps")
                    for kc in range(c.HK):
                        nc.tensor.matmul(
                            gps[:, :c.CAP],
                            w13k[kc][:, mg * 128:(mg + 1) * 128],
                            xg[:, kc, :],
                            start=(kc == 0), stop=(kc == c.HK - 1))
                    for kc in range(c.HK):
                        nc.tensor.matmul(
                            ups[:, :c.CAP],
                            w13k[kc][:, (c.IMK + mg) * 128:
                                     (c.IMK + mg + 1) * 128],
                            xg[:, kc, :],
                            start=(kc == 0), stop=(kc == c.HK - 1))
                    sg = SGP.tile([128, c.CAP], F32, tag="sg", name="sg")
                    nc.scalar.activation(sg[:], gps[:, :c.CAP], AF.Sigmoid)
                    nc.vector.tensor_tensor(sg[:], sg[:], gps[:, :c.CAP],
                                            op=ALU.mult)
                    nc.vector.tensor_tensor(actT[:, mg, :], sg[:],
                                            ups[:, :c.CAP], op=ALU.mult)

                w2k = []
                for ic in range(c.IMK):
                    t = W2P.tile([128, c.H], BF16, tag="w2t", name="w2t")
                    nc.sync.dma_start(
                        out=t[:],
                        in_=w2T.ap()[el, ic * 128:(ic + 1) * 128, :])
                    w2k.append(t)

                for sc in range(c.CAPC):
                    rank = el * c.CAPC + sc
                    for hf in range(2):
                        y_ps = PSY.tile([128, HH], F32, tag="y_ps", name="y_ps")
                        for no, nh in _chunks(HH, 512):
                            for ic in range(c.IMK):
                                nc.tensor.matmul(
                                    y_ps[:, no:no + nh],
                                    actT[:, ic, sc * 128:(sc + 1) * 128],
                                    w2k[ic][:, hf * HH + no:hf * HH + no + nh],
                                    start=(ic == 0), stop=(ic == c.IMK - 1))
                        nc.scalar.activation(
                            y_sb[:, rank, hf * HH:(hf + 1) * HH],
                            y_ps[:], AF.Copy,
                            scale=wslot_sb[:, rank:rank + 1])

        # =================================================================
        # Phase D: shared expert + combine, per H half
        # =================================================================
        g_tiles = _chunks(c.ISHL)
        with tc.tile_pool(name="shp", bufs=1) as SH, \
                tc.tile_pool(name="accp", bufs=1) as ACC, \
                tc.tile_pool(name="gthp", bufs=2) as GTH, \
                tc.tile_pool(name="tmpp", bufs=2) as TMP, \
                tc.tile_pool(name="ps_sh", bufs=2, space="PSUM") as PSSH, \
                tc.tile_pool(name="ps_ysh", bufs=2, space="PSUM") as PSYS:
            sgk = []
            for kc in range(c.HK):
                t = SH.tile([128, 2 * c.ISHL], BF16, tag=f"sgk{kc}", name=f"sgk{kc}")
                nc.sync.dma_start(out=t[:],
                                  in_=sguT.ap()[kc * 128:(kc + 1) * 128, :])
                sgk.append(t)

            actsh = []
            for gi, (mo, mh) in enumerate(g_tiles):
                gps = PSSH.tile([128, c.T], F32, tag="gsh_ps", name="gsh_ps")
                ups = PSSH.tile([128, c.T], F32, tag="gsh_ps", name="gsh_ps")
                for pso, tgt in ((mo, gps), (c.ISHL + mo, ups)):
                    for no, nh in _chunks(c.T, 512):
                        for kc in range(c.HK):
                            nc.tensor.matmul(
                                tgt[:mh, no:no + nh],
                                sgk[kc][:, pso:pso + mh],
                                xhiT_k[kc][:, no:no + nh],
                                start=(kc == 0), stop=(kc == c.HK - 1))
                sg = SH.tile([mh, c.T], F32, tag=f"sgsh{gi}", name=f"sgsh{gi}")
                nc.scalar.activation(sg[:], gps[:mh, :], AF.Sigmoid)
                nc.vector.tensor_tensor(sg[:], sg[:], gps[:mh, :],
                                        op=ALU.mult)
                at = SH.tile([mh, c.T], BF16, tag=f"actsh{gi}", name=f"actsh{gi}")
                nc.vector.tensor_tensor(at[:], sg[:], ups[:mh, :],
                                        op=ALU.mult)
                actsh.append(at)

            sdn_tiles = []
            for gi, (ko, kh) in enumerate(g_tiles):
                t = SH.tile([kh, c.H], BF16, tag=f"sdnt{gi}", name=f"sdnt{gi}")
                nc.sync.dma_start(out=t[:], in_=sdnT.ap()[ko:ko + kh, :])
                sdn_tiles.append(t)

            for hf in range(2):
                acc = ACC.tile([128, HB, c.T], F32, tag="acc", name="acc")
                for hb in range(HB):
                    hc = hf * HB + hb
                    ysh = PSYS.tile([128, c.T], F32, tag="ysh_ps", name="ysh_ps")
                    for no, nh in _chunks(c.T, 512):
                        for gi, (sdt, at) in enumerate(
                                zip(sdn_tiles, actsh)):
                            nc.tensor.matmul(
                                ysh[:, no:no + nh],
                                sdt[:, hc * 128:(hc + 1) * 128],
                                at[:, no:no + nh],
                                start=(gi == 0),
                                stop=(gi == len(g_tiles) - 1))
                    nc.scalar.copy(acc[:, hb, :], ysh[:])

                for tt in range(c.TT):
                    gt = GTH.tile([128, HB, 128 * NK], BF16, tag="gt", name="gt")
                    nc.gpsimd.dma_gather(
                        out_ap=gt[:],
                        in_ap=y_sb[:].rearrange("p r h -> p (r h)"),
                        idxs_ap=idxc_sb[tt][:],
                        num_idxs=128 * NK, num_idxs_reg=128 * NK,
                        elem_size=HH, transpose=True,
                        sbuf_tokens_per_rank=128,
                        sbuf_free_dim_per_rank=c.H * 2,
                        sbuf_free_dim_pad_per_rank=0,
                        sbuf_byte_offset=hf * c.H)
                    red = TMP.tile([128, HB, 128], F32, tag="red", name="red")
                    nc.vector.tensor_reduce(
                        red[:],
                        gt[:].rearrange("p b (t k) -> p b t k", k=NK),
                        AX.X, ALU.add)
                    nc.vector.tensor_tensor(
                        acc[:, :, tt * 128:(tt + 1) * 128],
                        acc[:, :, tt * 128:(tt + 1) * 128],
                        red[:], op=ALU.add)

                nc.sync.dma_start(
                    out=rs_in.ap()
                        .rearrange("(hc p) t -> p hc t", p=128)[
                            :, hf * HB:(hf + 1) * HB, :],
                    in_=acc[:])

        # =================================================================
        # Phase E: reduce-scatter + output
        # =================================================================
        if c.no_collective:
            nc.sync.dma_start(out=rs_out.ap(), in_=rs_in.ap()[:c.HO, :])
        else:
            nc.gpsimd.collective_compute(
                "ReduceScatter", ALU.add,
                ins=[rs_in.ap().opt()],
                outs=[rs_out.ap().opt()],
                replica_groups=[list(range(c.NC))],
            )
        with tc.tile_pool(name="outp", bufs=2) as OP:
            for po, ph in _chunks(c.HO):
                t = OP.tile([128, c.T], F32, tag="outt", name="outt")
                nc.sync.dma_start(out=t[:ph, :],
                                  in_=rs_out.ap()[po:po + ph, :])
                nc.sync.dma_start(out=out_ext.ap()[po:po + ph, :],
                                  in_=t[:ph, :])


# ---------------------------------------------------------------------------
# host side
# ---------------------------------------------------------------------------


def host_prep(cfg: Cfg, hidden_states, gate_w, w13, w2, shared_gu_w,
              shared_dn_w):
    c = cfg
    f32 = np.float32
    x = np.ascontiguousarray(np.asarray(hidden_states), dtype=f32)
    x_hi = x.astype(NPBF16)
    x_lo = (x - x_hi.astype(f32)).astype(NPBF16)
    gw = np.ascontiguousarray(np.asarray(gate_w), dtype=f32)
    gw_hi = gw.astype(NPBF16)
    gw_lo = (gw - gw_hi.astype(f32)).astype(NPBF16)

    pp = np.arange(128, dtype=f32)[:, None]
    com = {
        "xrow": np.ascontiguousarray(x_hi),
        "xhiT": np.ascontiguousarray(x_hi.T),
        "xloT": np.ascontiguousarray(x_lo.T),
        "gwhiT": np.ascontiguousarray(gw_hi.T),
        "gwloT": np.ascontiguousarray(gw_lo.T),
        "c_t1": (np.arange(128)[:, None] < np.arange(128)[None, :])
            .astype(f32),
        "c_ones": np.ones((128, 128), f32),
        "c_ident": np.eye(128, dtype=f32),
        "c_iota_tok": np.arange(c.TT, dtype=f32)[None, :] * 128 + pp + 1.0,
        "c_iota_slot": np.broadcast_to(
            np.arange(c.CAP, dtype=f32)[None, :], (128, c.CAP)).copy(),
        "c_iota_g": np.broadcast_to(
            (np.arange(c.E) // (c.E // c.G)).astype(f32)[None, :],
            (128, c.E)).copy(),
        "c_ebase": np.broadcast_to(
            (np.arange(c.EL, dtype=f32) * c.CAP)[None, :],
            (128, c.EL)).copy(),
    }

    w13 = np.asarray(w13); w2 = np.asarray(w2)
    shared_gu_w = np.asarray(shared_gu_w); shared_dn_w = np.asarray(shared_dn_w)

    in_maps = []
    for r in range(c.NC):
        m = dict(com)
        els = slice(r * c.EL, (r + 1) * c.EL)
        m["w13T"] = np.ascontiguousarray(
            np.transpose(w13[els].astype(f32), (0, 2, 1))).astype(NPBF16)
        m["w2T"] = np.ascontiguousarray(
            np.transpose(w2[els].astype(f32), (0, 2, 1))).astype(NPBF16)
        gsl = slice(r * c.ISHL, (r + 1) * c.ISHL)
        usl = slice(c.ISH + r * c.ISHL, c.ISH + (r + 1) * c.ISHL)
        sg = np.concatenate([shared_gu_w[gsl].astype(f32),
                             shared_gu_w[usl].astype(f32)], axis=0)
        m["sguT"] = np.ascontiguousarray(sg.T).astype(NPBF16)
        m["sdnT"] = np.ascontiguousarray(
            shared_dn_w[:, gsl].astype(f32).T).astype(NPBF16)
        m["c_iota_eloc"] = np.broadcast_to(
            (np.arange(c.EL, dtype=f32) + r * c.EL)[None, :],
            (128, c.EL)).copy()
        in_maps.append(m)
    return in_maps


def assemble(cfg: Cfg, results):
    chunks = [np.asarray(results[r]["out"], np.float32)
              for r in range(cfg.NC)]
    yT = np.concatenate(chunks, axis=0)
    return np.ascontiguousarray(yT.T)


_NC_CACHE = {}


def _get_nc(cfg: Cfg):
    if cfg not in _NC_CACHE:
        _NC_CACHE[cfg] = build_nc(cfg)
    return _NC_CACHE[cfg]


def kernel(**inputs) -> np.ndarray:
    from concourse.bass_utils import run_bass_kernel_spmd
    cfg = FULL
    nc = _get_nc(cfg)
    in_maps = host_prep(cfg, **inputs)
    res = run_bass_kernel_spmd(nc, in_maps, list(range(cfg.NC)))
    return assemble(cfg, res.results)
